# revision 7
# baseline (speedup 1.0000x reference)
"""DeTPP loss kernel for 8 TRN2 NeuronCores (batch-parallel SPMD Bass/Tile).

Strategy: shard along batch B (8 per core). Per core, on device:
  - one batched indirect-DMA row gather per group (SWDGE desc-gen
    ~0.34ns/desc + ~1us/instr): rows are [1024 bf16 logits (k-major) |
    24 f32 rec fields bitcast to 48 bf16 slots] = 2144B, so the rec
    windows ride the same descriptors as the logits
  - pick gathers (one per half): 4-contiguous-element runs from a
    c-major bf16 logits copy, offsets computed on DVE from gathered cat
  - ACT: exp per group (strided chunk reads, packed writes); DVE: bf16
    fold-tree + reduce for per-k sums over C; L1/cost assembly per half
    overlaps later groups' exp; lse joins perm-invariantly at the end
  - 24-permutation totals via PE transpose + block-diag matmul,
    segmented min-reduce, masked sum; host sums per-core (sum, count).
"""
import sys

sys.path.insert(0, '/opt/trn_rl_repo')

import itertools
import numpy as np
import ml_dtypes

BF16 = ml_dtypes.bfloat16

L, B, I, K, C = 1024, 64, 256, 4, 256
BS = B // 8            # batch per core
R = L * BS             # rows per core (8192), row id r = l*BS + b
N = I * BS             # gathered items per core (2048)
NT = N // 128          # 16 tiles; tile t holds item n = p*NT + t (p = partition)
KC = K * C             # 1024
RECW = 24              # rec fields (f32): t5(5) a4(4) c4(4) ot(4) oa(4) pad(3)
RECS = 2 * RECW        # rec slots in bf16 units (48)
ROWW = KC + RECS       # aug1 row width in bf16 slots (1072)
ROWF = ROWW // 2       # row width in f32 units (536)
F_T5, F_A4, F_C4, F_OT, F_OA = 512, 517, 521, 525, 529   # f32-elem offsets in row
PERMS = np.array(list(itertools.permutations(range(K))), dtype=np.int32)
NP_ = PERMS.shape[0]   # 24
GRP = [2, 2, 4, 4, 2, 2]   # tiles per row-gather group (h0 = g0+g1+g2)
GB = [sum(GRP[:i]) for i in range(len(GRP))]
HGRP = {0: [0, 1, 2], 1: [3, 4, 5]}


def _host_prep(core, time, amount, out_time, out_amount, out_cat_logits, cat,
               lengths, indices, consts):
    bsl = slice(core * BS, (core + 1) * BS)
    lg = out_cat_logits[:, bsl].astype(BF16)                   # (L, BS, K, C)
    t5 = np.stack([np.roll(time[:, bsl], -j, axis=0) for j in range(5)], axis=2)
    a4 = np.stack([np.roll(amount[:, bsl], -j, axis=0) for j in range(1, 5)], axis=2)
    c4 = np.stack([np.roll(cat[:, bsl], -j, axis=0) for j in range(1, 5)],
                  axis=2).astype(np.float32)
    rect = np.ascontiguousarray(np.concatenate(
        [t5, a4, c4, out_time[:, bsl], out_amount[:, bsl],
         np.zeros((L, BS, 3), np.float32)], axis=2), dtype=np.float32)
    aug1 = np.concatenate(
        [np.ascontiguousarray(lg).reshape(R, KC),
         rect.reshape(R, RECW).view(BF16)], axis=1)            # (R, 1072) bf16
    aug2 = np.ascontiguousarray(lg.transpose(0, 1, 3, 2)).reshape(R * KC, 1)
    idx = np.ascontiguousarray(indices[:, bsl]).reshape(128, NT)
    rowv = idx * BS + (np.arange(NT) % BS)[None, :].astype(np.int32)
    return {
        "aug1": aug1,
        "aug2": aug2,
        "rowv": rowv.astype(np.int32),
        "rowkc": (rowv * KC).astype(np.int32),
        "idxv": idx.astype(np.float32),
        "len_rep": np.tile(lengths[bsl][np.arange(NT) % BS].astype(np.float32),
                           (128, 1)),
        **consts,
    }


def _make_consts():
    pmat1 = np.zeros((K * K, NP_), np.float32)
    for p in range(NP_):
        for k in range(K):
            pmat1[k * K + PERMS[p, k], p] = 1.0
    pmat = np.zeros((128, 8 * NP_), np.float32)
    for tblk in range(8):
        pmat[tblk * 16:(tblk + 1) * 16, tblk * NP_:(tblk + 1) * NP_] = pmat1
    return {
        "pmat": pmat,
        "ident": np.eye(128, dtype=np.float32),
        "ones1": np.ones((128, 1), np.float32),
    }


def _build(nc, bass, mybir, tile):
    AP = bass.AP
    dt = mybir.dt
    Alu = mybir.AluOpType
    Act = mybir.ActivationFunctionType

    aug1 = nc.dram_tensor("aug1", [R, ROWW], dt.bfloat16, kind="ExternalInput")
    aug2 = nc.dram_tensor("aug2", [R * KC, 1], dt.bfloat16, kind="ExternalInput")
    rowv = nc.dram_tensor("rowv", [128, NT], dt.int32, kind="ExternalInput")
    rowkc = nc.dram_tensor("rowkc", [128, NT], dt.int32, kind="ExternalInput")
    idxv = nc.dram_tensor("idxv", [128, NT], dt.float32, kind="ExternalInput")
    len_rep = nc.dram_tensor("len_rep", [128, NT], dt.float32, kind="ExternalInput")
    pmat = nc.dram_tensor("pmat", [128, 8 * NP_], dt.float32, kind="ExternalInput")
    ident = nc.dram_tensor("ident", [128, 128], dt.float32, kind="ExternalInput")
    ones1 = nc.dram_tensor("ones1", [128, 1], dt.float32, kind="ExternalInput")
    out = nc.dram_tensor("out", [2, 1], dt.float32, kind="ExternalOutput")

    def vw(ap2, off, dims):
        # keep partition dim, replace free dims (strides/sizes in elements)
        a = ap2
        return AP(a.tensor, a.offset + off, [list(a.ap[0])] + [list(d) for d in dims])

    with tile.TileContext(nc) as tc:
        with (
            tc.tile_pool(name="consts", bufs=1) as cpool,
            tc.tile_pool(name="gbuf", bufs=1) as gpool,
            tc.tile_pool(name="ebuf", bufs=2) as epool,
            tc.tile_pool(name="fbuf", bufs=2) as fpool,
            tc.tile_pool(name="work", bufs=3) as wpool,
            tc.tile_pool(name="small", bufs=1) as spool,
            tc.tile_pool(name="psum", bufs=1, space="PSUM") as ppool,
        ):
            # ---- row ids first on sync (gate the gathers); rest parallel
            rowt = spool.tile([128, NT], dt.int32)
            nc.sync.dma_start(rowt[:], rowv.ap())
            rowk = spool.tile([128, NT], dt.int32)
            nc.sync.dma_start(rowk[:], rowkc.ap())
            idxf = spool.tile([128, NT], dt.float32)
            nc.scalar.dma_start(idxf[:], idxv.ap())
            c_len = cpool.tile([128, NT], dt.float32)
            nc.scalar.dma_start(c_len[:], len_rep.ap())
            c_pmat8 = cpool.tile([128, 8 * NP_], dt.float32)
            nc.sync.dma_start(c_pmat8[:], pmat.ap())
            c_id = cpool.tile([128, 128], dt.float32)
            nc.sync.dma_start(c_id[:], ident.ap())
            c_ones = cpool.tile([128, 1], dt.float32)
            nc.sync.dma_start(c_ones[:], ones1.ap())

            valid = spool.tile([128, NT], dt.float32)
            nc.vector.scalar_tensor_tensor(out=valid[:], in0=idxf[:], scalar=float(K),
                                           in1=c_len[:], op0=Alu.add, op1=Alu.is_lt)
            cnt = spool.tile([128, 1], dt.float32)
            nc.vector.tensor_reduce(out=cnt[:], in_=valid[:],
                                    axis=mybir.AxisListType.X, op=Alu.add)

            # ---- row gathers (rec fields ride along in each 2144B row)
            Gs = []
            for g in range(len(GRP)):
                G = gpool.tile([128, GRP[g] * ROWW], dt.bfloat16, tag=f"G{g}")
                nc.gpsimd.indirect_dma_start(
                    out=G[:], out_offset=None, in_=aug1.ap(),
                    in_offset=bass.IndirectOffsetOnAxis(
                        ap=rowt[:, GB[g]:GB[g] + GRP[g]], axis=0))
                Gs.append(G)

            # ---- pick offsets from gathered cat windows, one gather per half
            cati = spool.tile([128, NT * K], dt.int32)
            po = spool.tile([128, NT * K], dt.int32)
            pickt = spool.tile([128, NT * K * K], dt.bfloat16)

            def issue_picks(h):
                for g in HGRP[h]:
                    Gf = Gs[g][:].bitcast(dt.float32)
                    nc.vector.tensor_copy(
                        out=vw(cati[:], GB[g] * K, [[K, GRP[g]], [1, K]]),
                        in_=vw(Gf, F_C4, [[ROWF, GRP[g]], [1, K]]))
                t0 = h * (NT // 2)
                sl = slice(t0 * K, (t0 + NT // 2) * K)
                nc.vector.tensor_scalar(out=po[:, sl], in0=cati[:, sl],
                                        scalar1=K, scalar2=None, op0=Alu.mult)
                nc.vector.tensor_tensor(
                    out=vw(po[:], t0 * K, [[K, NT // 2], [1, K]]),
                    in0=vw(po[:], t0 * K, [[K, NT // 2], [1, K]]),
                    in1=vw(rowk[:], t0, [[1, NT // 2], [0, K]]), op=Alu.add)
                nc.gpsimd.indirect_dma_start(
                    out=pickt[:, t0 * K * K:(t0 + NT // 2) * K * K],
                    out_offset=None, in_=aug2.ap(),
                    in_offset=bass.IndirectOffsetOnAxis(ap=po[:, sl], axis=0))

            # ---- exp per group + per-(t,k) sums over C via bf16 fold tree
            s4all = spool.tile([128, NT * K], dt.float32)

            def exp_sums(g):
                nt = GRP[g]
                w = nt * KC
                E = epool.tile([128, w], dt.bfloat16, tag="E")
                nc.scalar.activation(
                    out=E[:].rearrange("p (t c) -> p t c", c=KC),
                    in_=vw(Gs[g][:], 0, [[ROWW, nt], [1, KC]]), func=Act.Exp)
                F = fpool.tile([128, w // 2], dt.bfloat16, tag="F")
                nk = nt * K
                nc.vector.tensor_tensor(
                    out=F[:].rearrange("p (s c) -> p s c", c=128),
                    in0=vw(E[:], 0, [[C, nk], [1, 128]]),
                    in1=vw(E[:], 128, [[C, nk], [1, 128]]), op=Alu.add)
                nc.vector.tensor_tensor(
                    out=vw(F[:], 0, [[128, nk], [1, 64]]),
                    in0=vw(F[:], 0, [[128, nk], [1, 64]]),
                    in1=vw(F[:], 64, [[128, nk], [1, 64]]), op=Alu.add)
                nc.vector.tensor_tensor(
                    out=vw(F[:], 0, [[128, nk], [1, 32]]),
                    in0=vw(F[:], 0, [[128, nk], [1, 32]]),
                    in1=vw(F[:], 32, [[128, nk], [1, 32]]), op=Alu.add)
                nc.vector.tensor_reduce(
                    out=s4all[:, GB[g] * K:(GB[g] + nt) * K],
                    in_=vw(F[:], 0, [[128, nk], [1, 32]]),
                    axis=mybir.AxisListType.X, op=Alu.add)

            # ---- per-half cost assembly + PE (cost = l1t + l1a - pick)
            acc = spool.tile([128, NT], dt.float32)
            costall = spool.tile([128, NT * K * K], dt.float32)
            d2 = spool.tile([128, NT * K * K], dt.float32)
            dtt = spool.tile([128, NT * K], dt.float32)

            def half_cost(h):
                for g in HGRP[h]:
                    nt = GRP[g]
                    Gf = Gs[g][:].bitcast(dt.float32)
                    co = GB[g] * K * K
                    nc.vector.tensor_tensor(
                        out=vw(dtt[:], GB[g] * K, [[K, nt], [1, K]]),
                        in0=vw(Gf, F_T5 + 1, [[ROWF, nt], [1, K]]),
                        in1=vw(Gf, F_T5, [[ROWF, nt], [0, K]]), op=Alu.subtract)
                    nc.vector.tensor_tensor(
                        out=vw(costall[:], co, [[K * K, nt], [K, K], [1, K]]),
                        in0=vw(Gf, F_OT, [[ROWF, nt], [1, K], [0, K]]),
                        in1=vw(dtt[:], GB[g] * K, [[K, nt], [0, K], [1, K]]),
                        op=Alu.subtract)
                    nc.vector.tensor_tensor(
                        out=vw(d2[:], co, [[K * K, nt], [K, K], [1, K]]),
                        in0=vw(Gf, F_OA, [[ROWF, nt], [1, K], [0, K]]),
                        in1=vw(Gf, F_A4, [[ROWF, nt], [0, K], [1, K]]),
                        op=Alu.subtract)
                t0 = h * (NT // 2)
                sl = slice(t0 * K * K, (t0 + NT // 2) * K * K)
                ts = NT // 2
                nc.vector.scalar_tensor_tensor(
                    out=costall[:, sl], in0=costall[:, sl], scalar=-1.0,
                    in1=costall[:, sl], op0=Alu.mult, op1=Alu.max)
                nc.vector.scalar_tensor_tensor(
                    out=d2[:, sl], in0=d2[:, sl], scalar=-1.0,
                    in1=d2[:, sl], op0=Alu.mult, op1=Alu.max)
                nc.vector.tensor_tensor(out=costall[:, sl], in0=costall[:, sl],
                                        in1=d2[:, sl], op=Alu.add)
                cv = vw(costall[:], t0 * K * K, [[K * K, ts], [K, K], [1, K]])
                nc.vector.tensor_tensor(
                    out=cv, in0=cv,
                    in1=vw(pickt[:], t0 * K * K, [[K * K, ts], [1, K], [K, K]]),
                    op=Alu.subtract)
                pT = ppool.tile([128, 128], dt.float32, tag=f"pT{h}")
                nc.tensor.transpose(out=pT[:], in_=costall[:, sl],
                                    identity=c_id[:])
                cT = spool.tile([128, 128], dt.float32, tag=f"cT{h}")
                nc.vector.tensor_copy(out=cT[:], in_=pT[:])
                ptot = ppool.tile([128, 8 * NP_], dt.float32, tag=f"ptot{h}")
                nc.tensor.matmul(out=ptot[:], lhsT=cT[:], rhs=c_pmat8[:],
                                 start=True, stop=True)
                mint8 = wpool.tile([128, 8], dt.float32, tag=f"mint{h}")
                nc.vector.tensor_reduce(
                    out=mint8[:], in_=ptot[:].rearrange("p (t q) -> p t q", q=NP_),
                    axis=mybir.AxisListType.X, op=Alu.min)
                nc.vector.tensor_tensor(out=acc[:, h * 8:(h + 1) * 8], in0=mint8[:],
                                        in1=valid[:, h * 8:(h + 1) * 8], op=Alu.mult)

            exp_sums(0)
            exp_sums(1)
            exp_sums(2)
            issue_picks(0)
            exp_sums(3)
            half_cost(0)
            exp_sums(4)
            issue_picks(1)
            exp_sums(5)
            half_cost(1)

            # ---- lse: single Ln at the end, perm-invariant join
            lnall = spool.tile([128, NT * K], dt.float32)
            nc.scalar.activation(out=lnall[:], in_=s4all[:], func=Act.Ln)
            sall = spool.tile([128, NT], dt.float32)
            nc.vector.tensor_reduce(
                out=sall[:], in_=lnall[:].rearrange("p (t k) -> p t k", k=K),
                axis=mybir.AxisListType.X, op=Alu.add)
            nc.vector.tensor_tensor(out=sall[:], in0=sall[:], in1=valid[:],
                                    op=Alu.mult)

            # ---- final reduction
            nc.vector.tensor_tensor(out=acc[:], in0=acc[:], in1=sall[:], op=Alu.add)
            pair = spool.tile([128, 2], dt.float32)
            nc.vector.tensor_reduce(out=pair[:, 0:1], in_=acc[:],
                                    axis=mybir.AxisListType.X, op=Alu.add)
            nc.vector.tensor_copy(out=pair[:, 1:2], in_=cnt[:])
            pf = ppool.tile([2, 1], dt.float32, tag="pf")
            nc.tensor.matmul(out=pf[:], lhsT=pair[:], rhs=c_ones[:],
                             start=True, stop=True)
            sb = spool.tile([2, 1], dt.float32)
            nc.vector.tensor_copy(out=sb[:], in_=pf[:])
            nc.sync.dma_start(out.ap(), sb[:])
    return nc


NCORES = 8
_COMPILED = {}


def _get_compiled():
    if "nc" not in _COMPILED:
        import concourse.bacc as bacc
        import concourse.bass as bass
        import concourse.mybir as mybir
        import concourse.tile as tile
        nc = bacc.Bacc("TRN2", target_bir_lowering=False, debug=False,
                       num_devices=NCORES)
        _build(nc, bass, mybir, tile)
        nc.compile()
        _COMPILED["nc"] = nc
    return _COMPILED["nc"]


def kernel(time, amount, out_time, out_amount, out_cat_logits, cat, lengths,
           indices):
    from concourse.bass_utils import run_bass_kernel_spmd

    time = np.asarray(time, dtype=np.float32)
    amount = np.asarray(amount, dtype=np.float32)
    out_time = np.asarray(out_time, dtype=np.float32)
    out_amount = np.asarray(out_amount, dtype=np.float32)
    out_cat_logits = np.asarray(out_cat_logits, dtype=np.float32)
    cat = np.asarray(cat, dtype=np.int32)
    lengths = np.asarray(lengths, dtype=np.int32)
    indices = np.asarray(indices, dtype=np.int32)

    nc = _get_compiled()
    consts = _make_consts()
    in_maps = [
        _host_prep(c, time, amount, out_time, out_amount, out_cat_logits, cat,
                   lengths, indices, consts)
        for c in range(NCORES)
    ]
    res = run_bass_kernel_spmd(nc, in_maps, core_ids=list(range(NCORES)))
    ls = sum(float(res.results[c]["out"][0, 0]) for c in range(NCORES))
    cn = sum(float(res.results[c]["out"][1, 0]) for c in range(NCORES))
    return np.float32(ls / (cn * K))
